# revision 9
# baseline (speedup 1.0000x reference)
"""Trainium2 Bass kernel for nn_DP_CAML_33646773797448 (sparse_attention).

Reference computation (per batch b):
    e      = embed_w[ids[b]]                       # (T, D)
    x      = e.T                                   # (D, T)
    h      = relu(conv1d(x, conv_w, pad=K-1) + b)  # (D, T')  T' = T + K - 1
    s      = U @ h                                 # (L, T')  raw scores
    attn   = softmax(s, axis=-1)
    z      = attn @ h.T                            # (L, D)
    logits = sum_d z * U + fc_bias                 # (L,)

Key identity used here:
    logits[l] = sum_t attn[l,t] * (U[l] . h[:,t]) = sum_t softmax(s)[l,t] * s[l,t]
i.e. the z-einsum and the final einsum collapse into a softmax-weighted mean
of the raw scores themselves. This halves tensor-engine work and removes all
transposes of the attention tensor.

Sharding: pure data-parallel over B (B == 8 == n_cores). Each core computes one
batch end-to-end; no collectives. Matmuls run in float32r (full-rate fp32 on
the PE for moving dims >= 256, ~tf32 precision).
"""

import numpy as np

import concourse.bass as bass
import concourse.tile as tile
from concourse import bacc
from concourse import mybir
from concourse.bass_utils import run_bass_kernel_spmd
from concourse.masks import make_identity

F32 = mybir.dt.float32
F32R = mybir.dt.float32r
I32 = mybir.dt.int32

# Problem shapes (hardcoded per contract)
VOCAB, L, D, K = 50000, 8921, 300, 10
B, T = 8, 2048
TP = T + K - 1            # 2057 conv output length
TP_PAD = TP + 1           # 2058: f32r matmuls need even moving widths
XW = T + 2 * (K - 1)      # 2066 padded input length
NTB = 17                  # gather blocks of 128 tokens (incl. 9+9 zero-pad via token 0)
T_G = NTB * 128           # 2176 gathered tokens (ids padded with token 0 = zero row)
DPAR = [128, 128, 44]     # D = 300 split into partition chunks
NDC = 3
LT = (L + 127) // 128     # 70 l-tiles (last one has 89 valid rows)
L_PAD = LT * 128          # 8960
D_PAD = NDC * 128         # 384

# conv t'-blocks (equal-ish, all >= 256 so f32r streams at full rate)
CONV_BLOCKS = [(0, 412), (412, 412), (824, 412), (1236, 412), (1648, 410)]
# scores: two PSUM tiles per l-tile; each matmul sub-block must sit inside one
# 512-fp32 PSUM bank (matmul outputs cannot cross bank boundaries)
# (tile_t0, stats_width, matmul sub-blocks); matmul widths are even (f32r
# requirement) and may overrun by 1 junk column that the stats never read
SCORE_TILES = [
    (0, 1024, [(0, 512), (512, 512)]),
    (1024, 1033, [(0, 512), (512, 512), (1024, 10)]),
]
NB = len(SCORE_TILES)  # accum columns per l-tile

_BUILT = {}


def _build_bass():
    nc = bacc.Bacc("TRN2", target_bir_lowering=False, debug=False)

    ids_d = nc.dram_tensor("ids", [T_G], I32, kind="ExternalInput").ap()
    emb_d = nc.dram_tensor("embed_w", [VOCAB, D], F32, kind="ExternalInput").ap()
    w_d = nc.dram_tensor("w_prep", [NDC, 128, K * D_PAD], F32R, kind="ExternalInput").ap()
    cb_d = nc.dram_tensor("cb_prep", [NDC, 128], F32, kind="ExternalInput").ap()
    ut_d = nc.dram_tensor("ut_prep", [LT, NDC, 128, 128], F32R, kind="ExternalInput").ap()
    fcb_d = nc.dram_tensor("fcb_prep", [LT, 128], F32, kind="ExternalInput").ap()
    out_d = nc.dram_tensor("out", [L], F32, kind="ExternalOutput").ap()

    with tile.TileContext(nc) as tc:
        _kernel_body(tc, ids_d, emb_d, w_d, cb_d, ut_d, fcb_d, out_d)
    nc.compile()
    return nc


def _kernel_body(tc, ids_d, emb_d, w_d, cb_d, ut_d, fcb_d, out_d):
    nc = tc.nc
    from contextlib import ExitStack

    ctx = ExitStack()
    with ctx:
        persist = ctx.enter_context(tc.tile_pool(name="persist", bufs=1))
        epool = ctx.enter_context(tc.tile_pool(name="epool", bufs=8))
        utpool = ctx.enter_context(tc.tile_pool(name="utpool", bufs=4))
        ppool = ctx.enter_context(tc.tile_pool(name="ppool", bufs=3))
        scrpool = ctx.enter_context(tc.tile_pool(name="scrpool", bufs=2))
        # PSUM: scores 2 tiles x 3 banks = 6 banks; conv/transpose share 2 x 1 bank
        sc_psum = ctx.enter_context(tc.tile_pool(name="sc_psum", bufs=2, space="PSUM"))
        cv_psum = ctx.enter_context(tc.tile_pool(name="cv_psum", bufs=2, space="PSUM"))

        # ---- constants / persistent tiles ----
        ids_sb = persist.tile([128, NTB], I32, name="ids_sb", tag="ids_sb")
        nc.sync.dma_start(out=ids_sb[:], in_=ids_d.rearrange("(n p) -> p n", p=128))

        cb_sb = persist.tile([128, NDC], F32, name="cb_sb", tag="cb_sb")
        nc.sync.dma_start(out=cb_sb[:], in_=cb_d.rearrange("c p -> p c"))

        fcb_sb = persist.tile([128, LT], F32, name="fcb_sb", tag="fcb_sb")
        nc.sync.dma_start(out=fcb_sb[:], in_=fcb_d.rearrange("n p -> p n"))

        ident = persist.tile([128, 128], F32, name="ident", tag="ident")
        make_identity(nc, ident[:])

        w_sb = []
        for ic in range(NDC):
            wt = persist.tile([128, K * D_PAD], F32R, name=f"w_sb{ic}", tag=f"w_sb{ic}")
            nc.sync.dma_start(out=wt[:], in_=w_d[ic])
            w_sb.append(wt)

        x_sb = []
        for ic in range(NDC):
            xt = persist.tile([128, T_G], F32R, name=f"x_sb{ic}", tag=f"x_sb{ic}")
            x_sb.append(xt)

        h_sb = []
        for oc in range(NDC):
            ht = persist.tile([128, TP_PAD], F32R, name=f"h_sb{oc}", tag=f"h_sb{oc}")
            h_sb.append(ht)

        # per-(l_tile, block) partial sums of p and p*s
        sp_all = persist.tile([128, LT * NB], F32, name="sp_all", tag="sp_all")
        sps_all = persist.tile([128, LT * NB], F32, name="sps_all", tag="sps_all")

        # ---- phase 1: embedding gather + transpose into x ----
        for tb in range(NTB):
            e_t = epool.tile([128, D], F32, name=f"e_t{tb}", tag="e_t")
            nc.gpsimd.indirect_dma_start(
                out=e_t[:],
                out_offset=None,
                in_=emb_d,
                in_offset=bass.IndirectOffsetOnAxis(ap=ids_sb[:, tb : tb + 1], axis=0),
            )
            for dc in range(NDC):
                dp = DPAR[dc]
                tp_ps = cv_psum.tile([128, 412], F32, name=f"tp{tb}_{dc}", tag="cvtp")
                nc.tensor.transpose(
                    out=tp_ps[:dp, :128],
                    in_=e_t[:, dc * 128 : dc * 128 + dp],
                    identity=ident[:],
                )
                nc.vector.tensor_copy(
                    out=x_sb[dc][:dp, tb * 128 : (tb + 1) * 128],
                    in_=tp_ps[:dp, :128],
                )

        # ---- phase 2: conv1d as matmul, fused bias+relu ----
        for oc in range(NDC):
            op = DPAR[oc]
            for t0, tw in CONV_BLOCKS:
                ps = cv_psum.tile([128, 412], F32, name=f"cv{oc}_{t0}", tag="cvtp")
                imm = 0
                for k in range(K):
                    for ic in range(NDC):
                        ip = DPAR[ic]
                        nc.tensor.matmul(
                            out=ps[:op, :tw],
                            lhsT=w_sb[ic][
                                :ip, k * D_PAD + oc * 128 : k * D_PAD + oc * 128 + op
                            ],
                            rhs=x_sb[ic][:ip, t0 + k : t0 + k + tw],
                            start=(imm == 0),
                            stop=(imm == K * NDC - 1),
                        )
                        imm += 1
                nc.scalar.activation(
                    out=h_sb[oc][:op, t0 : t0 + tw],
                    in_=ps[:op, :tw],
                    func=mybir.ActivationFunctionType.Relu,
                    bias=cb_sb[:op, oc : oc + 1],
                    scale=1.0,
                )

        # ---- phase 3: scores + online softmax-weighted-mean stats ----
        for lt in range(LT):
            ut_t = utpool.tile([128, NDC, 128], F32R, name=f"ut{lt}", tag="ut_t")
            nc.sync.dma_start(out=ut_t[:], in_=ut_d[lt].rearrange("c p l -> p c l"))
            for ti, (bt0, bw, subs) in enumerate(SCORE_TILES):
                ps = sc_psum.tile([128, 1034], F32, name=f"sc{lt}_{ti}", tag="sc")
                for s0, sw in subs:
                    for dc in range(NDC):
                        dp = DPAR[dc]
                        nc.tensor.matmul(
                            out=ps[:, s0 : s0 + sw],
                            lhsT=ut_t[:dp, dc, :],
                            rhs=h_sb[dc][:dp, bt0 + s0 : bt0 + s0 + sw],
                            start=(dc == 0),
                            stop=(dc == NDC - 1),
                        )
                col = lt * NB + ti
                p_t = ppool.tile([128, 1034], F32, name=f"p{lt}_{ti}", tag="p_t")
                nc.scalar.activation(
                    out=p_t[:, :bw],
                    in_=ps[:, :bw],
                    func=mybir.ActivationFunctionType.Exp,
                    accum_out=sp_all[:, col : col + 1],
                )
                sc_t = scrpool.tile([128, 1034], F32, name=f"ps{lt}_{ti}", tag="sc_t")
                nc.vector.scalar_tensor_tensor(
                    out=sc_t[:, :bw],
                    in0=p_t[:, :bw],
                    scalar=1.0,
                    in1=ps[:, :bw],
                    op0=mybir.AluOpType.mult,
                    op1=mybir.AluOpType.mult,
                    accum_out=sps_all[:, col : col + 1],
                )

        # ---- phase 4: combine partials, divide, add bias, write out ----
        den = persist.tile([128, LT], F32, name="den", tag="den")
        num = persist.tile([128, LT], F32, name="num", tag="num")
        rec = persist.tile([128, LT], F32, name="rec", tag="rec")
        logit = persist.tile([128, LT], F32, name="logit", tag="logit")
        nc.vector.tensor_reduce(
            out=den[:],
            in_=sp_all[:].rearrange("p (n t) -> p n t", t=NB),
            axis=mybir.AxisListType.X,
            op=mybir.AluOpType.add,
        )
        nc.vector.tensor_reduce(
            out=num[:],
            in_=sps_all[:].rearrange("p (n t) -> p n t", t=NB),
            axis=mybir.AxisListType.X,
            op=mybir.AluOpType.add,
        )
        nc.vector.reciprocal(out=rec[:], in_=den[:])
        nc.vector.tensor_tensor(
            out=logit[:], in0=num[:], in1=rec[:], op=mybir.AluOpType.mult
        )
        nc.vector.tensor_tensor(
            out=logit[:], in0=logit[:], in1=fcb_sb[:], op=mybir.AluOpType.add
        )

        n_full = L // 128  # 69 full l-tiles
        nc.sync.dma_start(
            out=out_d[0 : n_full * 128].rearrange("(n p) -> p n", p=128),
            in_=logit[:, :n_full],
        )
        tail = L - n_full * 128  # 89
        nc.sync.dma_start(
            out=out_d[n_full * 128 : L].rearrange("(p n) -> p n", n=1),
            in_=logit[:tail, n_full : n_full + 1],
        )


def _prep_inputs(ids, embed_w, conv_w, conv_b, U, fc_bias):
    ids = np.ascontiguousarray(np.asarray(ids, dtype=np.int32))
    embed_w = np.ascontiguousarray(np.asarray(embed_w, dtype=np.float32))
    conv_w = np.asarray(conv_w, dtype=np.float32)
    conv_b = np.asarray(conv_b, dtype=np.float32)
    U = np.asarray(U, dtype=np.float32)
    fc_bias = np.asarray(fc_bias, dtype=np.float32)

    # conv weights -> [ic, i_par, k, o_pad]; lhsT slice [i, o] per (k, oc)
    w_prep = np.zeros((NDC, 128, K, D_PAD), np.float32)
    cw = conv_w.transpose(1, 2, 0)  # (i, k, o)
    for ic in range(NDC):
        ip = DPAR[ic]
        w_prep[ic, :ip, :, :D] = cw[ic * 128 : ic * 128 + ip]
    w_prep = np.ascontiguousarray(w_prep.reshape(NDC, 128, K * D_PAD))

    cb_prep = np.zeros((NDC, 128), np.float32)
    cb_prep.reshape(-1)[:D] = conv_b

    Upad = np.zeros((L_PAD, D_PAD), np.float32)
    Upad[:L, :D] = U
    # [lt, dc, d_par, l_in_tile]
    ut_prep = np.ascontiguousarray(
        Upad.reshape(LT, 128, NDC, 128).transpose(0, 2, 3, 1)
    )

    fcb_prep = np.zeros((LT, 128), np.float32)
    fcb_prep.reshape(-1)[:L] = fc_bias

    common = {
        "embed_w": embed_w,
        "w_prep": w_prep,
        "cb_prep": cb_prep,
        "ut_prep": ut_prep,
        "fcb_prep": fcb_prep,
    }
    ids_pad = np.zeros((B, T_G), np.int32)
    ids_pad[:, K - 1 : K - 1 + T] = ids
    in_maps = [dict(common, ids=np.ascontiguousarray(ids_pad[b])) for b in range(B)]
    return in_maps


def get_bass():
    if "nc" not in _BUILT:
        _BUILT["nc"] = _build_bass()
    return _BUILT["nc"]


def kernel(ids, embed_w, conv_w, conv_b, U, fc_bias):
    nc = get_bass()
    in_maps = _prep_inputs(ids, embed_w, conv_w, conv_b, U, fc_bias)
    res = run_bass_kernel_spmd(nc, in_maps, list(range(B))).results
    return np.stack([res[b]["out"] for b in range(B)], axis=0)


# revision 11
# speedup vs baseline: 1.2919x; 1.2919x over previous
"""Trainium2 Bass kernel for nn_DP_CAML_33646773797448 (sparse_attention).

Reference computation (per batch b):
    e      = embed_w[ids[b]]                       # (T, D)
    x      = e.T                                   # (D, T)
    h      = relu(conv1d(x, conv_w, pad=K-1) + b)  # (D, T')  T' = T + K - 1
    s      = U @ h                                 # (L, T')  raw scores
    attn   = softmax(s, axis=-1)
    z      = attn @ h.T                            # (L, D)
    logits = sum_d z * U + fc_bias                 # (L,)

Key identity used here:
    logits[l] = sum_t attn[l,t] * (U[l] . h[:,t]) = sum_t softmax(s)[l,t] * s[l,t]
i.e. the z-einsum and the final einsum collapse into a softmax-weighted mean
of the raw scores themselves. This halves tensor-engine work and removes all
transposes of the attention tensor.

Sharding: pure data-parallel over B (B == 8 == n_cores). Each core computes one
batch end-to-end; no collectives. Matmuls run in float32r (full-rate fp32 on
the PE for moving dims >= 256, ~tf32 precision).
"""

import numpy as np

import concourse.bass as bass
import concourse.tile as tile
from concourse import bacc
from concourse import mybir
from concourse.bass_utils import run_bass_kernel_spmd
from concourse.masks import make_identity

F32 = mybir.dt.float32
F32R = mybir.dt.float32r
F16 = mybir.dt.float16
I32 = mybir.dt.int32

# Problem shapes (hardcoded per contract)
VOCAB, L, D, K = 50000, 8921, 300, 10
B, T = 8, 2048
TP = T + K - 1            # 2057 conv output length
TP_PAD = TP + 1           # 2058: f32r matmuls need even moving widths
XW = T + 2 * (K - 1)      # 2066 padded input length
NTB = 17                  # gather blocks of 128 tokens (incl. 9+9 zero-pad via token 0)
T_G = NTB * 128           # 2176 gathered tokens (ids padded with token 0 = zero row)
DPAR = [128, 128, 44]     # D = 300 split into partition chunks
NDC = 3
LT = (L + 127) // 128     # 70 l-tiles (last one has 89 valid rows)
L_PAD = LT * 128          # 8960
D_PAD = NDC * 128         # 384

# conv t'-blocks (equal-ish, all >= 256 so f32r streams at full rate)
CONV_BLOCKS = [(0, 412), (412, 412), (824, 412), (1236, 412), (1648, 410)]
# scores: two PSUM tiles per l-tile; each matmul sub-block must sit inside one
# 512-fp32 PSUM bank (matmul outputs cannot cross bank boundaries)
# (tile_t0, stats_width, matmul sub-blocks); matmul widths are even (f32r
# requirement) and may overrun by 1 junk column that the stats never read
SCORE_TILES = [
    (0, 1024, [(0, 512), (512, 512)]),
    (1024, 1033, [(0, 512), (512, 512), (1024, 10)]),
]
NB = len(SCORE_TILES)  # accum columns per l-tile

_BUILT = {}


def _build_bass():
    nc = bacc.Bacc("TRN2", target_bir_lowering=False, debug=False)

    ids_d = nc.dram_tensor("ids", [T_G], I32, kind="ExternalInput").ap()
    emb_d = nc.dram_tensor("embed_w", [VOCAB, D], F32, kind="ExternalInput").ap()
    w_d = nc.dram_tensor("w_prep", [NDC, 128, K * D_PAD], F16, kind="ExternalInput").ap()
    cb_d = nc.dram_tensor("cb_prep", [NDC, 128], F32, kind="ExternalInput").ap()
    ut_d = nc.dram_tensor("ut_prep", [LT, NDC, 128, 128], F16, kind="ExternalInput").ap()
    fcb_d = nc.dram_tensor("fcb_prep", [LT, 128], F32, kind="ExternalInput").ap()
    out_d = nc.dram_tensor("out", [L], F32, kind="ExternalOutput").ap()

    with tile.TileContext(nc) as tc:
        _kernel_body(tc, ids_d, emb_d, w_d, cb_d, ut_d, fcb_d, out_d)
    nc.compile()
    return nc


def _kernel_body(tc, ids_d, emb_d, w_d, cb_d, ut_d, fcb_d, out_d):
    nc = tc.nc
    from contextlib import ExitStack

    ctx = ExitStack()
    with ctx:
        persist = ctx.enter_context(tc.tile_pool(name="persist", bufs=1))
        epool = ctx.enter_context(tc.tile_pool(name="epool", bufs=8))
        utpool = ctx.enter_context(tc.tile_pool(name="utpool", bufs=4))
        ppool = ctx.enter_context(tc.tile_pool(name="ppool", bufs=3))
        scrpool = ctx.enter_context(tc.tile_pool(name="scrpool", bufs=2))
        # PSUM: scores 2 tiles x 3 banks = 6 banks; conv/transpose share 2 x 1 bank
        sc_psum = ctx.enter_context(tc.tile_pool(name="sc_psum", bufs=2, space="PSUM"))
        cv_psum = ctx.enter_context(tc.tile_pool(name="cv_psum", bufs=2, space="PSUM"))

        # ---- constants / persistent tiles ----
        ids_sb = persist.tile([128, NTB], I32, name="ids_sb", tag="ids_sb")
        nc.sync.dma_start(out=ids_sb[:], in_=ids_d.rearrange("(n p) -> p n", p=128))

        cb_sb = persist.tile([128, NDC], F32, name="cb_sb", tag="cb_sb")
        nc.sync.dma_start(out=cb_sb[:], in_=cb_d.rearrange("c p -> p c"))

        fcb_sb = persist.tile([128, LT], F32, name="fcb_sb", tag="fcb_sb")
        nc.sync.dma_start(out=fcb_sb[:], in_=fcb_d.rearrange("n p -> p n"))

        ident = persist.tile([128, 128], F32, name="ident", tag="ident")
        make_identity(nc, ident[:])

        w_sb = []
        for ic in range(NDC):
            wt = persist.tile([128, K * D_PAD], F16, name=f"w_sb{ic}", tag=f"w_sb{ic}")
            nc.sync.dma_start(out=wt[:], in_=w_d[ic])
            w_sb.append(wt)

        x_sb = []
        for ic in range(NDC):
            xt = persist.tile([128, T_G], F16, name=f"x_sb{ic}", tag=f"x_sb{ic}")
            x_sb.append(xt)

        h_sb = []
        for oc in range(NDC):
            ht = persist.tile([128, TP_PAD], F16, name=f"h_sb{oc}", tag=f"h_sb{oc}")
            h_sb.append(ht)

        # per-(l_tile, block) partial sums of p and p*s
        sp_all = persist.tile([128, LT * NB], F32, name="sp_all", tag="sp_all")
        sps_all = persist.tile([128, LT * NB], F32, name="sps_all", tag="sps_all")

        # ---- phase 1: embedding gather + transpose into x ----
        for tb in range(NTB):
            e_t = epool.tile([128, D], F32, name=f"e_t{tb}", tag="e_t")
            nc.gpsimd.indirect_dma_start(
                out=e_t[:],
                out_offset=None,
                in_=emb_d,
                in_offset=bass.IndirectOffsetOnAxis(ap=ids_sb[:, tb : tb + 1], axis=0),
            )
            for dc in range(NDC):
                dp = DPAR[dc]
                tp_ps = cv_psum.tile([128, 412], F32, name=f"tp{tb}_{dc}", tag="cvtp")
                nc.tensor.transpose(
                    out=tp_ps[:dp, :128],
                    in_=e_t[:, dc * 128 : dc * 128 + dp],
                    identity=ident[:],
                )
                nc.vector.tensor_copy(
                    out=x_sb[dc][:dp, tb * 128 : (tb + 1) * 128],
                    in_=tp_ps[:dp, :128],
                )

        # ---- phase 2: conv1d as matmul, fused bias+relu ----
        for oc in range(NDC):
            op = DPAR[oc]
            for t0, tw in CONV_BLOCKS:
                ps = cv_psum.tile([128, 412], F32, name=f"cv{oc}_{t0}", tag="cvtp")
                imm = 0
                for k in range(K):
                    for ic in range(NDC):
                        ip = DPAR[ic]
                        nc.tensor.matmul(
                            out=ps[:op, :tw],
                            lhsT=w_sb[ic][
                                :ip, k * D_PAD + oc * 128 : k * D_PAD + oc * 128 + op
                            ],
                            rhs=x_sb[ic][:ip, t0 + k : t0 + k + tw],
                            start=(imm == 0),
                            stop=(imm == K * NDC - 1),
                        )
                        imm += 1
                nc.scalar.activation(
                    out=h_sb[oc][:op, t0 : t0 + tw],
                    in_=ps[:op, :tw],
                    func=mybir.ActivationFunctionType.Relu,
                    bias=cb_sb[:op, oc : oc + 1],
                    scale=1.0,
                )

        # ---- phase 3: scores + online softmax-weighted-mean stats ----
        for lt in range(LT):
            ut_t = utpool.tile([128, NDC, 128], F16, name=f"ut{lt}", tag="ut_t")
            nc.sync.dma_start(out=ut_t[:], in_=ut_d[lt].rearrange("c p l -> p c l"))
            for ti, (bt0, bw, subs) in enumerate(SCORE_TILES):
                ps = sc_psum.tile([128, 1034], F32, name=f"sc{lt}_{ti}", tag="sc")
                for s0, sw in subs:
                    for dc in range(NDC):
                        dp = DPAR[dc]
                        nc.tensor.matmul(
                            out=ps[:, s0 : s0 + sw],
                            lhsT=ut_t[:dp, dc, :],
                            rhs=h_sb[dc][:dp, bt0 + s0 : bt0 + s0 + sw],
                            start=(dc == 0),
                            stop=(dc == NDC - 1),
                        )
                col = lt * NB + ti
                p_t = ppool.tile([128, 1034], F32, name=f"p{lt}_{ti}", tag="p_t")
                nc.scalar.activation(
                    out=p_t[:, :bw],
                    in_=ps[:, :bw],
                    func=mybir.ActivationFunctionType.Exp,
                    accum_out=sp_all[:, col : col + 1],
                )
                sc_t = scrpool.tile([128, 1034], F32, name=f"ps{lt}_{ti}", tag="sc_t")
                nc.vector.scalar_tensor_tensor(
                    out=sc_t[:, :bw],
                    in0=p_t[:, :bw],
                    scalar=1.0,
                    in1=ps[:, :bw],
                    op0=mybir.AluOpType.mult,
                    op1=mybir.AluOpType.mult,
                    accum_out=sps_all[:, col : col + 1],
                )

        # ---- phase 4: combine partials, divide, add bias, write out ----
        den = persist.tile([128, LT], F32, name="den", tag="den")
        num = persist.tile([128, LT], F32, name="num", tag="num")
        rec = persist.tile([128, LT], F32, name="rec", tag="rec")
        logit = persist.tile([128, LT], F32, name="logit", tag="logit")
        nc.vector.tensor_reduce(
            out=den[:],
            in_=sp_all[:].rearrange("p (n t) -> p n t", t=NB),
            axis=mybir.AxisListType.X,
            op=mybir.AluOpType.add,
        )
        nc.vector.tensor_reduce(
            out=num[:],
            in_=sps_all[:].rearrange("p (n t) -> p n t", t=NB),
            axis=mybir.AxisListType.X,
            op=mybir.AluOpType.add,
        )
        nc.vector.reciprocal(out=rec[:], in_=den[:])
        nc.vector.tensor_tensor(
            out=logit[:], in0=num[:], in1=rec[:], op=mybir.AluOpType.mult
        )
        nc.vector.tensor_tensor(
            out=logit[:], in0=logit[:], in1=fcb_sb[:], op=mybir.AluOpType.add
        )

        n_full = L // 128  # 69 full l-tiles
        nc.sync.dma_start(
            out=out_d[0 : n_full * 128].rearrange("(n p) -> p n", p=128),
            in_=logit[:, :n_full],
        )
        tail = L - n_full * 128  # 89
        nc.sync.dma_start(
            out=out_d[n_full * 128 : L].rearrange("(p n) -> p n", n=1),
            in_=logit[:tail, n_full : n_full + 1],
        )


def _prep_inputs(ids, embed_w, conv_w, conv_b, U, fc_bias):
    ids = np.ascontiguousarray(np.asarray(ids, dtype=np.int32))
    embed_w = np.ascontiguousarray(np.asarray(embed_w, dtype=np.float32))
    conv_w = np.asarray(conv_w, dtype=np.float32)
    conv_b = np.asarray(conv_b, dtype=np.float32)
    U = np.asarray(U, dtype=np.float32)
    fc_bias = np.asarray(fc_bias, dtype=np.float32)

    # conv weights -> [ic, i_par, k, o_pad]; lhsT slice [i, o] per (k, oc)
    w_prep = np.zeros((NDC, 128, K, D_PAD), np.float32)
    cw = conv_w.transpose(1, 2, 0)  # (i, k, o)
    for ic in range(NDC):
        ip = DPAR[ic]
        w_prep[ic, :ip, :, :D] = cw[ic * 128 : ic * 128 + ip]
    w_prep = np.ascontiguousarray(
        w_prep.reshape(NDC, 128, K * D_PAD).astype(np.float16)
    )

    cb_prep = np.zeros((NDC, 128), np.float32)
    cb_prep.reshape(-1)[:D] = conv_b

    Upad = np.zeros((L_PAD, D_PAD), np.float32)
    Upad[:L, :D] = U
    # [lt, dc, d_par, l_in_tile]
    ut_prep = np.ascontiguousarray(
        Upad.reshape(LT, 128, NDC, 128).transpose(0, 2, 3, 1).astype(np.float16)
    )

    fcb_prep = np.zeros((LT, 128), np.float32)
    fcb_prep.reshape(-1)[:L] = fc_bias

    common = {
        "embed_w": embed_w,
        "w_prep": w_prep,
        "cb_prep": cb_prep,
        "ut_prep": ut_prep,
        "fcb_prep": fcb_prep,
    }
    ids_pad = np.zeros((B, T_G), np.int32)
    ids_pad[:, K - 1 : K - 1 + T] = ids
    in_maps = [dict(common, ids=np.ascontiguousarray(ids_pad[b])) for b in range(B)]
    return in_maps


def get_bass():
    if "nc" not in _BUILT:
        _BUILT["nc"] = _build_bass()
    return _BUILT["nc"]


def kernel(ids, embed_w, conv_w, conv_b, U, fc_bias):
    nc = get_bass()
    in_maps = _prep_inputs(ids, embed_w, conv_w, conv_b, U, fc_bias)
    res = run_bass_kernel_spmd(nc, in_maps, list(range(B))).results
    return np.stack([res[b]["out"] for b in range(B)], axis=0)
